# revision 28
# baseline (speedup 1.0000x reference)
"""Trainium2 Bass kernel for nn_DensityFunction (wavefunction density + FD gradient norm).

Math:
  S = tril(dm,-1) + tril(dm,-1).T + diag(diag(dm))
  density(c) = sum_p x_p * (S@x)_p,  x_p(n) = sh_p(n) * exp(-alpha_p * |c_n - a_{center_p}|^2)
  sh_p = prod_i (c_i - a_i)^{pw_i},  pw = SYM_TABLE[sym], total degree <= 2
  g = sqrt( sum_i ((density(c + dh e_i) - density(c - dh e_i)) / (2 dh))^2 ),  dh = 1e-8 (f32!)

Implementation strategy:
  Both sh_p and the exponent -alpha_p d2_p are degree-<=2 polynomials in c, so with the
  10-monomial feature basis F = [1, x, y, z, x2, y2, z2, xy, xz, yz] they become one
  matmul  [sh; G] = W.T @ F  with host-precomputed W [10, 2P].  The 7 density
  evaluations (base + 6 perturbed for the central differences) each run:
     W.T@F (PE, 4x row-group-packed K=10) -> E=exp(G) (ACT) -> x=sh*E (DVE)
     -> block-symmetric quadratic form x'Sx = x0'S00x0 + x1'S11x1 + 2x0'S01x1
        (3 PE matmuls + 3 DVE muls + 2 gpsimd adds)
     -> ones-reduce over the 128 partitions (PE) -> per-eval density row
  All arithmetic is float32: the FD gradient with dh=1e-8 is dominated by f32
  rounding noise by construction (matching the reference's own numerics); any
  lower-precision matmul path would round the 1e-8 perturbations away entirely.
  Data-parallel over grid points: each of the 8 cores handles N/8 = 4096 points.
  Host does the final FD combine in numpy f32, bit-faithful to the reference ops.
"""

import numpy as np

# Problem constants (hardcoded per harness contract)
N_FULL = 32768
N_ATOMS = 32
P = 256
N_CORES = 8
N_LOC = N_FULL // N_CORES  # 4096
CHUNK = 512
N_CHUNKS = N_LOC // CHUNK  # 8
N_EVALS = 7  # base, +x, -x, +y, -y, +z, -z

DH = np.float32(1e-8)

SYM_TABLE = np.array(
    [[0, 0, 0],
     [1, 0, 0], [0, 1, 0], [0, 0, 1],
     [2, 0, 0], [0, 2, 0], [0, 0, 2],
     [1, 1, 0], [1, 0, 1], [0, 1, 1]], dtype=np.int32)

MONO_IDX = {(0, 0, 0): 0, (1, 0, 0): 1, (0, 1, 0): 2, (0, 0, 1): 3,
            (2, 0, 0): 4, (0, 2, 0): 5, (0, 0, 2): 6,
            (1, 1, 0): 7, (1, 0, 1): 8, (0, 1, 1): 9}

_COMPILED = {}


def _poly_mul(p1, p2):
    out = {}
    for m1, c1 in p1.items():
        for m2, c2 in p2.items():
            m = tuple(a + b for a, b in zip(m1, m2))
            out[m] = out.get(m, 0.0) + c1 * c2
    return out


def _build_W(ac_sel, pw, alpha):
    """W [10, 2P] f32: columns 0..P-1 = sh coefficients, P..2P-1 = exp-argument coeffs."""
    n_prim = ac_sel.shape[0]
    W = np.zeros((10, 2 * n_prim), np.float64)
    for p in range(n_prim):
        poly = {(0, 0, 0): 1.0}
        for ax in range(3):
            a = float(ac_sel[p, ax])
            k = int(pw[p, ax])
            if k == 0:
                continue
            e1 = tuple(1 if i == ax else 0 for i in range(3))
            e2 = tuple(2 if i == ax else 0 for i in range(3))
            fac = {e1: 1.0, (0, 0, 0): -a} if k == 1 else {e2: 1.0, e1: -2.0 * a, (0, 0, 0): a * a}
            poly = _poly_mul(poly, fac)
        for m, c in poly.items():
            W[MONO_IDX[m], p] = c
        a = ac_sel[p].astype(np.float64)
        al = float(alpha[p])
        W[0, n_prim + p] = -al * float(a @ a)
        W[1:4, n_prim + p] = 2.0 * al * a
        W[4:7, n_prim + p] = -al
    return W.astype(np.float32)


def _feats(c):
    """Monomial features [10, N] f32 of coordinates [N, 3] f32."""
    cx = c[:, 0].astype(np.float32)
    cy = c[:, 1].astype(np.float32)
    cz = c[:, 2].astype(np.float32)
    return np.stack([np.ones_like(cx), cx, cy, cz,
                     cx * cx, cy * cy, cz * cz,
                     cx * cy, cx * cz, cy * cz]).astype(np.float32)


def _build_bass():
    import concourse.bacc as bacc
    import concourse.mybir as mybir
    from concourse.tile import TileContext

    f32 = mybir.dt.float32

    # Bacc (not raw Bass): its compile() runs move_matmul_waits_to_ldweights +
    # generate_event_semaphores, which split multi-sem waits down to the 1-wait
    # per-instruction limit this walrus build enforces.
    nc = bacc.Bacc("TRN2", target_bir_lowering=False, debug=False)
    F_d = nc.declare_dram_parameter("F", [N_EVALS, 10, N_LOC], f32, isOutput=False)
    W_d = nc.declare_dram_parameter("W", [128, 128], f32, isOutput=False)
    S_d = nc.declare_dram_parameter("S", [P, P], f32, isOutput=False)
    C_d = nc.declare_dram_parameter("C", [128, 128], f32, isOutput=False)
    R_d = nc.declare_dram_parameter("R", [128, N_EVALS * N_EVALS], f32, isOutput=False)
    D_d = nc.declare_dram_parameter("D", [N_EVALS, N_LOC], f32, isOutput=True)

    Exp = mybir.ActivationFunctionType.Exp

    with TileContext(nc) as tc:
        with (
            tc.tile_pool(name="const", bufs=1) as cpool,
            tc.tile_pool(name="fpool", bufs=1) as fpool,
            tc.tile_pool(name="work", bufs=3) as wpool,
            tc.tile_pool(name="ps1", bufs=1, space="PSUM") as ps1pool,
            tc.tile_pool(name="ps2", bufs=1, space="PSUM") as ps2pool,
            tc.tile_pool(name="psp", bufs=1, space="PSUM") as psppool,
        ):
            # W row-packed on host: rows 32m+r hold W[r, 128m:128(m+1)] so the
            # four K=10 mm1 matmuls live in distinct PE row-groups concurrently
            W_sb = cpool.tile([128, 128], f32, tag="W")
            nc.gpsimd.dma_start(W_sb[:], W_d[:])
            # Symmetry split: p = x0'S00x0 + x1'S11x1 + 2 x0'S01x1 -> 3 matmuls
            # S00 lhsT = S[0:128,0:128]; S11 lhsT = S[128:,128:];
            # cross lhsT = 2*S[0:128,128:256] (host-provided in C_d)
            S00_sb = cpool.tile([128, 128], f32, tag="S00")
            nc.gpsimd.dma_start(S00_sb[:], S_d[0:128, 0:128])
            S11_sb = cpool.tile([128, 128], f32, tag="S11")
            nc.gpsimd.dma_start(S11_sb[:], S_d[128:256, 128:256])
            C_sb = cpool.tile([128, 128], f32, tag="C")
            nc.gpsimd.dma_start(C_sb[:], C_d[:])
            R_sb = cpool.tile([128, N_EVALS * N_EVALS], f32, tag="R")
            nc.gpsimd.dma_start(R_sb[:], R_d[:])

            # F replicated into the four row-groups (partitions 32m..32m+9),
            # DMA'd in quarter-column waves (eval-major within a wave) so the
            # first chunks' slices of every eval land before PE needs them
            F_sb = [fpool.tile([106, N_LOC], f32, tag=f"F{e}", name=f"F{e}")
                    for e in range(N_EVALS)]
            QUARTER = N_LOC // 4
            for q in range(4):
                qlo = q * QUARTER
                for e in range(N_EVALS):
                    for m in range(4):
                        nc.sync.dma_start(
                            F_sb[e][32 * m:32 * m + 10, qlo:qlo + QUARTER],
                            F_d[e][:, qlo:qlo + QUARTER])

            D_sb = cpool.tile([N_EVALS, N_LOC], f32, tag="D")

            # Deferred reduce, carried across chunk boundaries: the p-bank for a
            # chunk is allocated only when its first reduce is emitted, which is
            # one eval AFTER the previous chunk's last reduce + copy were
            # emitted - so the psum slot tracker sees every prior accessor.
            cur_bank = [None]

            def emit_reduce(rlo, e, rhs):
                if e == 0:
                    cur_bank[0] = psppool.tile(
                        [N_EVALS, CHUNK], f32, tag="p", name=f"p{rlo}")
                pb = cur_bank[0]
                # ones-reduce over the 128 partitions, accumulated into
                # p_bank row e via R's eval-selector columns
                nc.tensor.matmul(
                    pb[:], R_sb[:, e * N_EVALS:(e + 1) * N_EVALS], rhs[:],
                    start=(e == 0), stop=(e == N_EVALS - 1),
                    skip_group_check=True)
                if e == N_EVALS - 1:
                    nc.scalar.copy(D_sb[:, rlo:rlo + CHUNK], pb[:])
                    nc.sync.dma_start(
                        D_d[:, rlo:rlo + CHUNK], D_sb[:, rlo:rlo + CHUNK])

            pending = None  # (lo, e, usum) deferred so PE needn't wait on the
            # DVE->gpsimd usum chain before starting the next eval's matmuls
            for ch in range(N_CHUNKS):
                lo = ch * CHUNK
                for e in range(N_EVALS):
                    # mm1: [sh(2 blocks); G(2 blocks)] = W.T @ F, 4 row-packed
                    # matmuls; G blocks first so ACT exp overlaps the sh matmuls
                    shg = [None] * 4
                    for mb in (2, 3, 0, 1):
                        t = ps1pool.tile([128, CHUNK], f32, tag=f"shg{mb}")
                        nc.tensor.matmul(
                            t[:], W_sb[32 * mb:32 * mb + 10, :],
                            F_sb[e][32 * mb:32 * mb + 10, lo:lo + CHUNK],
                            tile_position=(32 * mb, 0))
                        shg[mb] = t
                    # x = sh * exp(G)
                    x = []
                    for b in range(2):
                        E_t = wpool.tile([128, CHUNK], f32, tag=f"E{b}")
                        nc.scalar.activation(E_t[:], shg[2 + b][:], Exp)
                        x_t = wpool.tile([128, CHUNK], f32, tag=f"x{b}")
                        nc.vector.tensor_mul(x_t[:], E_t[:], shg[b][:])
                        x.append(x_t)
                    # block-symmetric quadratic form: three matmuls T0, T1, Cx
                    u = []
                    for name, lhs, rhs_x, mul_x in (
                        ("t0", S00_sb, x[0], x[0]),
                        ("t1", S11_sb, x[1], x[1]),
                        ("cx", C_sb, x[0], x[1]),
                    ):
                        t = ps2pool.tile([128, CHUNK], f32, tag=name)
                        nc.tensor.matmul(t[:], lhs[:], rhs_x[:])
                        u_t = wpool.tile([128, CHUNK], f32, tag=f"u_{name}")
                        nc.vector.tensor_mul(u_t[:], mul_x[:], t[:])
                        u.append(u_t)
                    # fold the three terms (gpsimd - DVE is busier)
                    us0 = wpool.tile([128, CHUNK], f32, tag="us0")
                    nc.gpsimd.tensor_add(us0[:], u[0][:], u[1][:])
                    usum = wpool.tile([128, CHUNK], f32, tag="usum")
                    nc.gpsimd.tensor_add(usum[:], us0[:], u[2][:])
                    if pending is not None:
                        emit_reduce(*pending)
                    pending = (lo, e, usum)
            emit_reduce(*pending)

    nc.compile()
    return nc


def kernel(coords, atom_coords, centers, exp, sym, dm):
    from concourse.bass_utils import run_bass_kernel_spmd

    coords = np.asarray(coords, dtype=np.float32)
    atom_coords = np.asarray(atom_coords, dtype=np.float32)
    centers = np.asarray(centers).astype(np.int64)
    alpha = np.asarray(exp, dtype=np.float32)
    sym = np.asarray(sym).astype(np.int64)
    dm = np.asarray(dm, dtype=np.float32)

    # Host precompute (cheap O(P) / O(N) work)
    lower = np.tril(dm, -1)
    S = (lower + lower.T + np.diag(np.diag(dm))).astype(np.float32)
    ac_sel = atom_coords[centers]
    pw = SYM_TABLE[sym]
    W = _build_W(ac_sel, pw, alpha)
    Wp = np.zeros((128, 128), np.float32)  # row-packed for the 4 PE row-groups
    for m in range(4):
        Wp[32 * m:32 * m + 10, :] = W[:, 128 * m:128 * (m + 1)]

    # 7 coordinate sets, replicating the reference's f32 perturbation arithmetic
    c_sets = [coords]
    for i in range(3):
        cb = coords.copy()
        cb[:, i] = cb[:, i] + DH
        c_sets.append(cb)
        cb2 = cb.copy()
        cb2[:, i] = cb2[:, i] - np.float32(2.0) * DH
        c_sets.append(cb2)
    F_all = np.stack([_feats(c) for c in c_sets])  # [7, 10, N_FULL] f32

    # Reduction selector: R[:, e*7+j] = 1 if j == e else 0 (ones column per eval)
    R = np.zeros((128, N_EVALS * N_EVALS), np.float32)
    for e in range(N_EVALS):
        R[:, e * N_EVALS + e] = 1.0

    if "nc" not in _COMPILED:
        _COMPILED["nc"] = _build_bass()
    nc = _COMPILED["nc"]

    core_ids = list(range(N_CORES))
    in_maps = []
    for i in core_ids:
        sl = slice(i * N_LOC, (i + 1) * N_LOC)
        in_maps.append({
            "F": np.ascontiguousarray(F_all[:, :, sl]),
            "W": Wp,
            "S": S,
            "C": np.ascontiguousarray((np.float32(2.0) * S[0:128, 128:256])),
            "R": R,
        })

    res = run_bass_kernel_spmd(nc, in_maps, core_ids)
    _COMPILED["last_result"] = res  # exec_time_ns/profile for the test harness
    D = np.concatenate([res.results[i]["D"] for i in core_ids], axis=1)  # [7, N_FULL]

    # FD combine on host, bit-faithful to the reference's f32 ops
    p = D[0].copy()
    two_dh = np.float32(2.0) * DH
    g = np.zeros(N_FULL, np.float32)
    for i in range(3):
        diff = ((D[1 + 2 * i] - D[2 + 2 * i]) / two_dh).astype(np.float32)
        g = (g + np.square(diff)).astype(np.float32)
    g = np.sqrt(g)
    return (p, g)
